# revision 22
# baseline (speedup 1.0000x reference)
"""CheapBiMamba3D Trainium2 kernel (8-core SPMD, D-axis sharded).

Math identities used (exact, no approximation):
  - in_proj is 1x1 over (h,w) and only the ::4 subsample feeds the mamba,
    so in_proj runs on the 32x32 token grid only.
  - nearest-upsample(out_proj(feat)) == out_proj applied per upsampled voxel,
    so the final conv runs on the small grid and the upsample happens via a
    repeat-read matmul AP (W) + repeated DMA stores (H).
  - ln folded into mamba in_w:  xz = (in_w*ln_w) @ t_hat + in_w@ln_b
  - softplus(u) = ln(exp(u)+1)   (ACT Exp then Ln with bias=1)
  - silu(v) = v * sigmoid(v)     (ACT Sigmoid + DVE mult)
  - dA_s = exp(A[:,s] * dt)      (ACT Exp with per-partition scale AP)
Layout: state tiles are (128 partitions = (slice n in {0,1}) x (di in 0..63),
free = 1024 tokens of that slice). The dst axis (16) is the tile index s.
"""
import sys
import functools
from contextlib import ExitStack

import numpy as np

for _p in ("/opt/trn_rl_repo", "/root/.axon_site/_ro/trn_rl_repo"):
    if _p not in sys.path:
        sys.path.insert(0, _p)

import ml_dtypes
import concourse.bass as bass
import concourse.tile as tile
from concourse import mybir

F32 = mybir.dt.float32
F16 = mybir.dt.float16
BF16 = mybir.dt.bfloat16
OUT_SCALE = 1024.0  # output written as scaled fp16; host divides back
AF = mybir.ActivationFunctionType
ALU = mybir.AluOpType
BF16_NP = ml_dtypes.bfloat16

# problem constants
B, C, D, H, W = 1, 256, 16, 128, 128
CR, DST, DCONV, EXPAND, S = 32, 16, 4, 2, 4
DI = EXPAND * CR          # 64
DTR = 2
NCORES = 8
DPC = D // NCORES         # 2 slices per core
HS = WS = 32              # token grid per slice
L = HS * WS               # 1024 tokens per slice
NT = DPC * L              # 2048 tokens per core
NCHUNK = NT // 128        # 16 token chunks


# ----------------------------------------------------------------- blob packing
class BlobSpec:
    """Static column layout of the packed constants blob (one per dtype)."""

    def __init__(self):
        self.items = {}   # name -> (rows, col0, cols)
        self.ncols = 0

    def add(self, name, rows, cols, row0=0):
        self.items[name] = (rows, self.ncols, cols, row0)
        self.ncols += cols

    def pack(self, arrays, np_dtype):
        buf = np.zeros((128, self.ncols), np_dtype)
        for name, arr in arrays.items():
            rows, c0, cols, row0 = self.items[name]
            a = np.asarray(arr, np.float32)
            assert a.shape == (rows, cols), (name, a.shape, (rows, cols))
            buf[row0 : row0 + rows, c0 : c0 + cols] = a.astype(np_dtype)
        return buf

    def sl(self, tile_ap, name):
        rows, c0, cols, row0 = self.items[name]
        return tile_ap[row0 : row0 + rows, c0 : c0 + cols]


def _blob_specs():
    fb = BlobSpec()
    fb.add("ident", 128, 128)           # PE-transpose identity
    fb.add("w_inT0", 128, CR)           # w_in.T rows 0:128
    fb.add("w_inT1", 128, CR)           # w_in.T rows 128:256
    fb.add("w_outT0_0", CR, 128)        # w_out.T cols 0:128   (lhsT K=CR M=128)
    fb.add("w_outT1_0", CR, 128)
    fb.add("w_outT0_1", CR, 128, row0=32)
    fb.add("w_outT1_1", CR, 128, row0=32)
    fb.add("eps", 128, 1)
    for d in ("mf", "mb"):
        fb.add(d + "_A", 128, DST)      # A[di,s] tiled over n -> (128, 16)
        fb.add(d + "_dtb", 128, 1)
        fb.add(d + "_convw", 128, DCONV)
        fb.add(d + "_convb", 128, 1)
        fb.add(d + "_biasx2", 128, 1)   # x-half of in_w@ln_b, tiled both halves
        fb.add(d + "_biasz2", 128, 1)   # z-half of in_w@ln_b, tiled both halves
        fb.add(d + "_inwT", CR, 128)    # (in_w*ln_w).T : lhsT K=CR M=128

    bb = BlobSpec()
    bb.add("I128", 128, 128)            # identity bf16 (y reduction lhsT)
    for s in range(DST):
        bb.add(f"selB{s}", 128, 128)    # Esel for B_s broadcast
        bb.add(f"selC{s}", 128, 128)
    for d in ("mf", "mb"):
        bb.add(d + "_diagD", 128, 128)             # diag(D) tiled over n
        bb.add(d + "_xprojT0", DI, DTR + 2 * DST)  # xproj_w.T (64, 34)
        bb.add(d + "_xprojT1", DI, DTR + 2 * DST, row0=64)
        bb.add(d + "_dtwT0", DTR, DI)              # dt_w.T (2, 64)
        bb.add(d + "_dtwT1", DTR, DI, row0=64)
        bb.add(d + "_outwT0", DI, CR)              # (0.5*out_w).T (64, 32)
        bb.add(d + "_outwT1", DI, CR, row0=64)
    return fb, bb


FB, BB = _blob_specs()

# dbc_sb layout rows: n0 at 0 (dtraw 0:2, B 2:18, C 18:34), n1 at 64.
_DBC_N1 = 64


def _host_blobs(w):
    """w: dict of the full-problem weight arrays (numpy float32)."""
    f = {}
    b = {}
    f["ident"] = np.eye(128, dtype=np.float32)
    w_inT = np.asarray(w["w_in"]).T  # (256, 32)
    f["w_inT0"] = w_inT[:128]
    f["w_inT1"] = w_inT[128:]
    w_outT = OUT_SCALE * np.asarray(w["w_out"]).T  # (32, 256), fp16-scaled
    for nn in range(2):
        f[f"w_outT0_{nn}"] = w_outT[:, :128]
        f[f"w_outT1_{nn}"] = w_outT[:, 128:]
    f["eps"] = np.full((128, 1), 1e-5, np.float32)
    b["I128"] = np.eye(128, dtype=np.float32)
    for s in range(DST):
        mB = np.zeros((128, 128), np.float32)
        mB[2 + s, 0:DI] = 1.0
        mB[_DBC_N1 + 2 + s, DI:128] = 1.0
        b[f"selB{s}"] = mB
        mC = np.zeros((128, 128), np.float32)
        mC[2 + DST + s, 0:DI] = 1.0
        mC[_DBC_N1 + 2 + DST + s, DI:128] = 1.0
        b[f"selC{s}"] = mC
    ln_w = np.asarray(w["ln_w"])
    ln_b = np.asarray(w["ln_b"])
    for d in ("mf", "mb"):
        A = -np.exp(np.asarray(w[d + "_A_log"]))          # (64, 16)
        f[d + "_A"] = np.tile(A, (2, 1))
        f[d + "_dtb"] = np.tile(np.asarray(w[d + "_dt_b"]), 2)[:, None]
        f[d + "_convw"] = np.tile(np.asarray(w[d + "_conv_w"]), (2, 1))
        f[d + "_convb"] = np.tile(np.asarray(w[d + "_conv_b"]), 2)[:, None]
        in_w = np.asarray(w[d + "_in_w"])                 # (128, 32)
        bxz = in_w @ ln_b
        f[d + "_biasx2"] = np.tile(bxz[0:DI], 2)[:, None]
        f[d + "_biasz2"] = np.tile(bxz[DI:], 2)[:, None]
        b[d + "_diagD"] = np.diag(np.tile(np.asarray(w[d + "_D"]), 2))
        f[d + "_inwT"] = (in_w * ln_w[None, :]).T          # (32, 128)
        for nn in range(2):
            b[f"{d}_xprojT{nn}"] = np.asarray(w[d + "_xproj_w"]).T
            b[f"{d}_dtwT{nn}"] = np.asarray(w[d + "_dt_w"]).T
            b[f"{d}_outwT{nn}"] = 0.5 * np.asarray(w[d + "_out_w"]).T
    return FB.pack(f, np.float32), BB.pack(b, BF16_NP)


# -------------------------------------------------------------- waitsplit pass
def _split_multi_waits(nc):
    """walrus codegen accepts at most ONE sync wait per instruction; hoist
    extras onto standalone same-engine InstEventSemaphore waits."""
    trash = nc._waitsplit_sem
    n_split = 0
    for fn in nc.m.functions:
        for bb in fn.blocks:
            out = []
            for inst in bb.instructions:
                si = getattr(inst, "sync_info", None)
                if (
                    si is not None
                    and len(si.on_wait) > 1
                    and getattr(inst, "engine", None) is not None
                    and not isinstance(inst, mybir.InstEventSemaphore)
                ):
                    waits = list(si.on_wait)
                    for w in waits[:-1]:
                        ab = mybir.InstEventSemaphore(
                            name=nc.get_next_instruction_name(), ins=[], outs=[])
                        ab.engine = inst.engine
                        upd = mybir.SyncUpdate(
                            sync_type="semaphore", id=trash.num,
                            ant_name=trash.name, update_mode="sem-inc",
                            update_value=1)
                        ab.sync_info = mybir.SyncInfo(on_wait=[w], on_update=[upd])
                        out.append(ab)
                        n_split += 1
                    si.on_wait[:] = [waits[-1]]
                out.append(inst)
            bb.instructions[:] = out
    return n_split


# ----------------------------------------------------------------- device build
def build_nc(structured=True):
    nc = bass.Bass()
    nc._waitsplit_sem = nc.alloc_semaphore("waitsplit-trash")
    xs_d = nc.dram_tensor("xs", [C, NT], F32, kind="ExternalInput")
    fb_d = nc.dram_tensor("fblob", [128, FB.ncols], F32, kind="ExternalInput")
    bb_d = nc.dram_tensor("bblob", [128, BB.ncols], BF16, kind="ExternalInput")
    out_d = nc.dram_tensor("out", [C, DPC, H, W], F16, kind="ExternalOutput")

    with tile.TileContext(nc) as tc, ExitStack() as ctx:
        P = ctx.enter_context  # shorthand
        wpool = P(tc.tile_pool(name="weights", bufs=1))
        spool = P(tc.tile_pool(name="state", bufs=1))

        # ---- loads
        xs0 = wpool.tile([128, NT], F32, tag="xs0")
        xs1 = wpool.tile([128, NT], F32, tag="xs1")
        fbt = wpool.tile([128, FB.ncols], F32, tag="fbt")
        bbt = wpool.tile([128, BB.ncols], BF16, tag="bbt")
        nc.gpsimd.dma_start(xs0[:], xs_d[0:128, :])
        nc.gpsimd.dma_start(xs1[:], xs_d[128:256, :])
        nc.gpsimd.dma_start(fbt[:], fb_d[:])
        nc.gpsimd.dma_start(bbt[:], bb_d[:])
        fsl = lambda name: FB.sl(fbt, name)
        bsl = lambda name: BB.sl(bbt, name)

        # PE wait-absorbers (matmul may carry only one sync wait)
        with tc.tile_pool(name="touch", bufs=1, space="PSUM") as tp:
            scr = tp.tile([1, 1], F32)
            for t_ in (xs0, xs1, fbt, bbt):
                nc.tensor.matmul(scr[:], t_[0:1, 0:1], t_[0:1, 0:1],
                                 start=True, stop=True)

        # ---- phase 1: tok = w_in' @ x per token chunk, LN stats, transpose
        tokn = spool.tile([CR, NT], F32, tag="tokn")      # channel-major tokens
        tokn_r = spool.tile([CR, NT], F32, tag="tokn_r")  # per-slice reversed
        stats = spool.tile([128, 2 * NCHUNK], F32, tag="stats")
        rstd = spool.tile([128, NCHUNK], F32, tag="rstd")
        lnv = spool.tile([128, NCHUNK], F32, tag="lnv")
        with (
            tc.tile_pool(name="p1psum", bufs=1, space="PSUM") as pp,
            tc.tile_pool(name="p1tp", bufs=2, space="PSUM") as ptp,
            tc.tile_pool(name="p1sb", bufs=3) as sp,
        ):
            tokp = pp.tile([128, CR * NCHUNK], F32)   # all 16 chunks, 1 bank
            for k in range(NCHUNK):
                cs = slice(128 * k, 128 * (k + 1))
                nc.tensor.matmul(tokp[:, CR * k : CR * (k + 1)],
                                 xs0[:, cs], fsl("w_inT0"), start=True, stop=False)
                nc.tensor.matmul(tokp[:, CR * k : CR * (k + 1)],
                                 xs1[:, cs], fsl("w_inT1"), start=False, stop=True)
            for k in range(NCHUNK):
                st6 = sp.tile([128, 6], F32, tag="st6")
                nc.vector.bn_stats(st6[:], tokp[:, CR * k : CR * (k + 1)])
                nc.vector.bn_aggr(stats[:, 2 * k : 2 * k + 2], st6[:])
            # rstd = exp(-0.5*ln(var+eps)) over all chunks at once
            nc.scalar.activation(lnv[:], stats[:, 1 : 2 * NCHUNK : 2], AF.Ln,
                                 bias=fsl("eps"), scale=1.0)
            nc.scalar.activation(rstd[:], lnv[:], AF.Exp, scale=-0.5)
            for k in range(NCHUNK):
                tn = sp.tile([128, CR], F32, tag="tn")
                nc.vector.tensor_scalar(tn[:], tokp[:, CR * k : CR * (k + 1)],
                                        stats[:, 2 * k : 2 * k + 1],
                                        rstd[:, k : k + 1],
                                        ALU.subtract, ALU.mult)
                tptile = ptp.tile([CR, 128], F32, tag="tpt")
                nc.tensor.transpose(tptile[:], tn[:], fsl("ident"))
                nc.scalar.copy(tokn[:, 128 * k : 128 * (k + 1)], tptile[:])
        for n in range(DPC):
            ts = slice(L * n, L * (n + 1))
            nc.scalar.copy(tokn_r[:, ts], tokn[:, ts][:, ::-1])

        # ---- phase 2+: per direction
        dirs = (("mf", tokn), ("mb", tokn_r))
        sigctx = {}

        # 2a: xz matmul, z-gate sigmoid, x evac, conv, conv sigmoid [sigmoid set]
        for d, tsrc in dirs:
            xsx = spool.tile([128, 3 + L], F32, tag=d + "_xsx")
            sz = spool.tile([128, L], BF16, tag=d + "_sz")
            xsil = spool.tile([128, L], BF16, tag=d + "_xsil")
            nc.vector.memset(xsx[:, 0:3], 0.0)
            with (
                tc.tile_pool(name=d + "xz", bufs=2, space="PSUM") as pxz,
                tc.tile_pool(name=d + "cv", bufs=2) as cvp,
            ):
                for n in range(DPC):
                    ts = slice(L * n, L * (n + 1))
                    rows = slice(DI * n, DI * (n + 1))
                    xzp = pxz.tile([128, L], F32, tag="xzp")
                    for j in range(2):
                        nc.tensor.matmul(xzp[:, 512 * j : 512 * (j + 1)],
                                         fsl(d + "_inwT"),
                                         tsrc[:, ts][:, 512 * j : 512 * (j + 1)],
                                         start=True, stop=True)
                    # x half -> xsx rows (with ln_b fold bias)
                    nc.scalar.activation(xsx[rows, 3 : 3 + L], xzp[0:DI, :],
                                         AF.Identity,
                                         bias=fsl(d + "_biasx2")[rows, 0:1])
                    # z half: sz = (z + bias_z) * sigmoid(z + bias_z)
                    sg = cvp.tile([128, L], F32, tag="sg")
                    nc.scalar.activation(sg[rows, :], xzp[DI:128, :], AF.Sigmoid,
                                         bias=fsl(d + "_biasz2")[rows, 0:1])
                    nc.vector.scalar_tensor_tensor(
                        sz[rows, :], xzp[DI:128, :],
                        fsl(d + "_biasz2")[rows, 0:1], sg[rows, :],
                        ALU.add, ALU.mult)
                # depthwise causal conv along t (both slices together)
                acc = cvp.tile([128, L], F32, tag="acc")
                nc.vector.tensor_scalar(acc[:], xsx[:, 0:L],
                                        fsl(d + "_convw")[:, 0:1], None, ALU.mult)
                for k in (1, 2, 3):
                    nc.vector.scalar_tensor_tensor(
                        acc[:], xsx[:, k : k + L],
                        fsl(d + "_convw")[:, k : k + 1], acc[:],
                        ALU.mult, ALU.add)
                sgc = cvp.tile([128, L], F32, tag="sgc")
                nc.scalar.activation(sgc[:], acc[:], AF.Sigmoid,
                                     bias=fsl(d + "_convb"))
                nc.vector.scalar_tensor_tensor(
                    xsil[:], acc[:], fsl(d + "_convb"), sgc[:],
                    ALU.add, ALU.mult)
            sigctx[d] = (xsx, sz, xsil)

        # 2b: xproj, dt (softplus via exp/ln), dA, scan core, gate [nle set]
        ym = {}
        for d, _ in dirs:
            xsx, sz, xsil = sigctx[d]
            dbc = spool.tile([128, L], BF16, tag=d + "_dbc")
            nc.gpsimd.memset(dbc[:], 0.0)
            dt = spool.tile([128, L], F32, tag=d + "_dt")
            eu = spool.tile([128, L], F32, tag=d + "_eu")
            dtx = spool.tile([128, L], BF16, tag=d + "_dtx")
            ymt = spool.tile([128, L], BF16, tag=d + "_ym")
            with (
                tc.tile_pool(name=d + "py", bufs=1, space="PSUM") as pyy,
                tc.tile_pool(name=d + "sc", bufs=3) as scp,
            ):
                ppj = tc.alloc_tile_pool(name=d + "pj", bufs=1, space="PSUM")
                pbc = None
                for n in range(DPC):
                    rows = slice(DI * n, DI * (n + 1))
                    dbcp = ppj.tile([DTR + 2 * DST, L], F32, tag="pj")
                    for j in range(2):
                        js = slice(512 * j, 512 * (j + 1))
                        nc.tensor.matmul(dbcp[:, js], bsl(f"{d}_xprojT{n}"),
                                         xsil[rows, js], start=True, stop=True)
                    nc.scalar.copy(dbc[_DBC_N1 * n : _DBC_N1 * n + DTR + 2 * DST, :],
                                   dbcp[:])
                dtp = ppj.tile([128, L], F32, tag="pj")
                for n in range(DPC):
                    rows = slice(DI * n, DI * (n + 1))
                    for j in range(2):
                        js = slice(512 * j, 512 * (j + 1))
                        nc.tensor.matmul(
                            dtp[rows, js], bsl(f"{d}_dtwT{n}"),
                            dbc[_DBC_N1 * n : _DBC_N1 * n + DTR, js],
                            start=True, stop=True)
                # dt = ln(exp(u)+1), u = dtp + dt_b
                nc.scalar.activation(eu[:], dtp[:], AF.Exp,
                                     bias=fsl(d + "_dtb"))
                nc.scalar.activation(dt[:], eu[:], AF.Ln, bias=1.0)
                nc.gpsimd.tensor_tensor(dtx[:], dt[:], xsil[:], ALU.mult)
                ppj.release()
                pbc = tc.alloc_tile_pool(name=d + "bc", bufs=3, space="PSUM")

                yp = pyy.tile([128, L], F32)   # y accumulator (2 banks)
                nc.tensor.matmul(yp[:, 0:512], bsl(d + "_diagD"),
                                 xsil[:, 0:512], start=True, stop=False)
                nc.tensor.matmul(yp[:, 512:1024], bsl(d + "_diagD"),
                                 xsil[:, 512:1024], start=True, stop=False)
                pend = []
                # structured A (A[:,s] = -(s+1)): dA_s = r^(s+1); first half
                # from ACT Exp (persisted), second half as off-chain Pool
                # products of two finished first-half tiles.
                dA_keep = {}
                _PROD = {8: (3, 4), 9: (4, 4), 10: (4, 5), 11: (5, 5),
                         12: (5, 6), 13: (6, 6), 14: (6, 7), 15: (7, 7)}
                for s in range(DST):
                    if structured and s >= 8:
                        a_, b_ = _PROD[s]
                        dA = scp.tile([128, L], BF16, tag="dA")
                        nc.gpsimd.tensor_tensor(dA[:], dA_keep[a_][:],
                                                dA_keep[b_][:], ALU.mult)
                    elif structured:
                        dA = spool.tile([128, L], BF16, tag=f"{d}_dA{s}")
                        nc.scalar.activation(dA[:], dt[:], AF.Exp,
                                             scale=fsl(d + "_A")[:, s : s + 1])
                        dA_keep[s] = dA
                    else:
                        dA = scp.tile([128, L], BF16, tag="dA")
                        nc.scalar.activation(dA[:], dt[:], AF.Exp,
                                             scale=fsl(d + "_A")[:, s : s + 1])
                    bbp = pbc.tile([128, L], F32, tag="bcp")
                    nc.tensor.matmul(bbp[:, 0:512], bsl(f"selB{s}"),
                                     dbc[:, 0:512], start=True, stop=True)
                    nc.tensor.matmul(bbp[:, 512:1024], bsl(f"selB{s}"),
                                     dbc[:, 512:1024], start=True, stop=True)
                    cbp = pbc.tile([128, L], F32, tag="bcp")
                    nc.tensor.matmul(cbp[:, 0:512], bsl(f"selC{s}"),
                                     dbc[:, 0:512], start=True, stop=True)
                    nc.tensor.matmul(cbp[:, 512:1024], bsl(f"selC{s}"),
                                     dbc[:, 512:1024], start=True, stop=True)
                    dBx = scp.tile([128, L], BF16, tag="dBx")
                    hs = scp.tile([128, L], BF16, tag="hs")
                    hc = scp.tile([128, L], BF16, tag="hc")
                    if s % 4 == 0:
                        # direct psum-operand path on DVE
                        nc.vector.tensor_tensor(dBx[:], dtx[:], bbp[:], ALU.mult)
                        nc.vector.tensor_tensor_scan(hs[:], dA[:], dBx[:], 0.0,
                                                     ALU.mult, ALU.add)
                        nc.vector.tensor_tensor(hc[:], hs[:], cbp[:], ALU.mult)
                    else:
                        # ACT evacuates broadcasts to sbuf bf16; TTs run 2x
                        # on DVE or on the Pool engine (3-way balance)
                        bbs = scp.tile([128, L], BF16, tag="bbs")
                        cbs = scp.tile([128, L], BF16, tag="cbs")
                        nc.scalar.copy(bbs[:], bbp[:])
                        nc.scalar.copy(cbs[:], cbp[:])
                        eng = nc.vector if s % 2 == 1 else nc.gpsimd
                        eng.tensor_tensor(dBx[:], dtx[:], bbs[:], ALU.mult)
                        nc.vector.tensor_tensor_scan(hs[:], dA[:], dBx[:], 0.0,
                                                     ALU.mult, ALU.add)
                        eng.tensor_tensor(hc[:], hs[:], cbs[:], ALU.mult)
                    pend.append(hc)
                    if len(pend) > 1:
                        hcp = pend.pop(0)
                        for j in range(2):
                            js = slice(512 * j, 512 * (j + 1))
                            nc.tensor.matmul(yp[:, js], bsl("I128"), hcp[:, js],
                                             start=False, stop=False)
                hcp = pend.pop(0)
                for j in range(2):
                    js = slice(512 * j, 512 * (j + 1))
                    nc.tensor.matmul(yp[:, js], bsl("I128"), hcp[:, js],
                                     start=False, stop=True)
                # gate
                nc.vector.tensor_tensor(ymt[:], yp[:], sz[:], ALU.mult)
                pbc.release()
            ym[d] = ymt

        # flip backward ym back to forward time
        ymb_f = spool.tile([128, L], BF16, tag="ymb_f")
        nc.scalar.copy(ymb_f[:], ym["mb"][:][:, ::-1])

        # ---- out_proj (0.5 folded in out_wT) + combine directions
        feat = spool.tile([2 * CR, L], F32, tag="feat")  # rows (n, r)
        with tc.tile_pool(name="po", bufs=2, space="PSUM") as po:
            for n in range(DPC):
                rows = slice(DI * n, DI * (n + 1))
                yop = po.tile([CR, L], F32, tag="yop")
                for j in range(2):
                    js = slice(512 * j, 512 * (j + 1))
                    nc.tensor.matmul(yop[:, js], bsl(f"mf_outwT{n}"),
                                     ym["mf"][rows, js], start=True, stop=False)
                    nc.tensor.matmul(yop[:, js], bsl(f"mb_outwT{n}"),
                                     ymb_f[rows, js], start=False, stop=True)
                nc.scalar.copy(feat[CR * n : CR * (n + 1), :], yop[:])

        # ---- final 1x1 conv to C channels with nearest upsample
        # feat rows (n, r); per (n, chalf): psum (128c, 2048) = 16 h' rows of
        # 128 upsampled w; evac to sbuf; DMA 4x with h-repeat.
        with (
            tc.tile_pool(name="pf", bufs=2, space="PSUM") as pf,
            tc.tile_pool(name="os", bufs=3) as osb,
        ):
            for n in range(DPC):
                frows = feat[CR * n : CR * (n + 1), :]
                mv = frows.rearrange("p (h w) -> p h w", h=HS)
                mv = mv.unsqueeze(3).broadcast_to([CR, HS, WS, S])
                for ch in range(2):
                    for hb in range(2):  # h' blocks of 16
                        op = pf.tile([128, 2048], F32, tag="op")
                        for q in range(4):  # 4 h' rows per matmul (N=512)
                            hrow = 16 * hb + 4 * q
                            nc.tensor.matmul(
                                op[:, 512 * q : 512 * (q + 1)],
                                fsl(f"w_outT{ch}_{n}"),
                                mv[:, hrow : hrow + 4, :, :],
                                start=True, stop=True)
                        ot = osb.tile([128, 2048], F16, tag="ot")
                        if (n + ch + hb) % 2 == 0:
                            nc.scalar.copy(ot[:], op[:])
                        else:
                            nc.vector.tensor_copy(ot[:], op[:])
                        src = ot[:].rearrange("p (h w) -> p h w", h=16)
                        for j in range(S):
                            h0 = S * 16 * hb + j
                            nc.sync.dma_start(
                                out_d[128 * ch : 128 * (ch + 1), n,
                                      h0 : h0 + 61 : S, :],
                                src)
    return nc


# ----------------------------------------------------------------- entry points
@functools.lru_cache(maxsize=2)
def _built(structured=True):
    nc = build_nc(structured)
    _split_multi_waits(nc)
    return nc


def _a_structured(w):
    ref = -np.tile(np.arange(1, DST + 1, dtype=np.float32), (DI, 1))
    return all(
        np.allclose(-np.exp(np.asarray(w[d + "_A_log"])), ref, rtol=1e-5)
        for d in ("mf", "mb")
    )


def prep_inputs(inputs):
    x = np.asarray(inputs["x"])  # (1, 256, 16, 128, 128)
    xsub = x[0][:, :, ::S, ::S]  # (256, 16, 32, 32)
    fblob, bblob = _host_blobs(inputs)
    in_maps = []
    for c in range(NCORES):
        shard = np.ascontiguousarray(
            xsub[:, DPC * c : DPC * (c + 1)]).reshape(C, NT)
        in_maps.append({"xs": shard, "fblob": fblob, "bblob": bblob})
    return in_maps


def kernel(**inputs):
    from concourse.bass_utils import run_bass_kernel_spmd

    nc = _built(_a_structured(inputs))
    in_maps = prep_inputs(inputs)
    res = run_bass_kernel_spmd(nc, in_maps, list(range(NCORES)))
    parts = [res.results[c]["out"] for c in range(NCORES)]
    out = np.concatenate(parts, axis=1).astype(np.float32)  # (256,16,128,128)
    out *= np.float32(1.0 / OUT_SCALE)
    return out[None]
